# revision 48
# baseline (speedup 1.0000x reference)
"""Trainium2 Bass kernel for nn_AtomicKANLayer (v8).

Math: y[b,o] = sum_{i,d} fupn((x[b,i]-centers[d])*compression[d]) * coeffs[i,o,d]
with fupn evaluated via its Fourier series.  Key structure exploited:

* The series coefficients decay superpolynomially: NK=7 terms leave 6.6e-4
  relative error (gate is 2e-2), so the trig features of EIGHT 512-element
  element-groups pack into 121 partitions (14 trig rows + 1 x row per group,
  one shared ones row) and every elementwise pass runs on a 512-wide free dim
  instead of 2048.
* With centers on the exact grid cent_d = d/16-1 and compression 16, the
  series argument z_d = u - d (u = 16x+16) is 3-PERIODIC in d: only three
  distinct series values S_c per element (c = d mod 3).  The per-(elem,d)
  basis is S_{d mod 3} * [z_d^2 <= a^2]; the d-replication of the three
  S_c values is folded into the ws matmul weights (group blocks of
  33 z cols + 33 S cols), keeping every elementwise AP <= 3D.

* The 2.1MB coefficient tensor is quantized to fp8e3 (e3m4, 4 mantissa
  bits) on the host, halving the HBM-bound coefficient stream that gates
  the output contraction.  Measured end-to-end relative error 1.24e-2
  (gate 2e-2); the host rescales y by CO_SCALE/co_scale afterwards.

The PE p-state governor only reaches full clock (~0.42ns/col) after several
microseconds of sustained full-array matmul power; K=1 matmuls do not count,
and the tile scheduler reorders same-engine work.  So every known PE wait
window is filled with full-K dummy matmuls into a scratch PSUM bank, with
all PE instructions chained by ordering-only deps, so the 33-matmul output
contraction runs at full clock almost immediately.  Warm counts are tuned
down from (10,9,1,5): the excess warms tripped the activity governor's
50%-duty cap (ham k=4/8 window) squarely across the y contraction.

Device pipeline per core (data-parallel over batch, 32 rows of B=256):
  DMA: sync queue: uq+pq first, then coeff chunks 0,1 and the y store;
  gpsimd queue: w_s, xs, coeff chunks 2,3 (small critical transfers are
  never queued behind the 2.2MB coeff stream).
  1. q[p,c] = theta_p/2pi * x + off_p  (exact split-fp16 matmul, K=33)  [PE]
  2. f = q - round(q) via fp32 magic-constant (ts add/sub, tt sub)     [DVE]
  3. trig[0:112] = Sin(2pi f) -> fp16, two 256-col chunks              [ACT]
  4. S_j = trig-colslice.T @ w_s  -> [128 i, 4 groups x (z_d, S_d)],
     two matmuls per j into bank-aligned halves of a 2-bank PSUM tile  [PE]
  5. msq = z^2                                                         [ACT]
     bas = (msq <= a^2) * S_d                                          [DVE]
  6. y = sum_d bas_d.T @ co_d (33 fp16 x fp8e3 matmuls, fp32 PSUM)      [PE]
  7. y_s = y_a + y_b -> DMA out (rows are (j,g)-permuted; host fixes)
"""
import sys

sys.path.insert(0, "/opt/trn_rl_repo")

import numpy as np

F16 = np.float16
B, I, O, D = 256, 128, 256, 33
NCORES = 8
BLOC = B // NCORES          # 32 batch rows per core
ROWS = BLOC * I             # 4096 flattened (b, i) elements per core
NG = 8                      # element groups packed along partitions
FD = ROWS // NG             # 512 free-dim columns per group
NK = 5                      # Fourier terms kept (of reference's 100)
GROWS = 2 * NK              # trig rows per group (cos+sin)
NPART = NG * GROWS + NG + 1  # 112 trig + 8 xh + 1 ones = 121
N_ORDER, NPROD = 1, 10
A_SUP = (N_ORDER + 2) / 2.0  # support half-width a = 1.5
CVAL = 16.0                  # compression (asserted on host)
MAGIC = float(np.float32(1.5 * 2**23))
TWO_PI = float(2 * np.pi)
QCH = 2                      # q -> sin chain chunks (of FD)
CO_CHUNKS = 4
CO_SCALE = 256.0             # bas = basis/CO_SCALE (keeps fp16 bas normal)
CO_FP8 = True                # co in fp8e3 (e3m4): halves the HBM-bound
                             # coeff stream; measured end-to-end 1.23e-2
CO_QMAX = 8.0                # quantize co with absmax -> 8.0
# PE warm-up dummies: (n_before_q, n_q_to_S, n_sin2_gap, n_S_to_y) 256-col
# full-K matmuls (the p-state governor ignores low-power work)
WARM = (8, 6, 1, 3)
DSPLIT = 16                  # y accumulator split point over d
USE_MOD = False

_PROG = None


def _build_program():
    import concourse.bacc as bacc
    import concourse.tile as tile
    from concourse import mybir

    f32 = mybir.dt.float32
    f16 = mybir.dt.float16
    codt = mybir.dt.float8e3 if CO_FP8 else f16
    Alu = mybir.AluOpType
    Act = mybir.ActivationFunctionType

    nc = bacc.Bacc("TRN2", target_bir_lowering=False, debug=False,
                   num_devices=NCORES)
    uqpq_d = nc.dram_tensor("uqpq", [33, FD + 128], f16, kind="ExternalInput")
    xs_d = nc.dram_tensor("xs", [NG + 1, FD], f16, kind="ExternalInput")
    ws_d = nc.dram_tensor("ws", [NPART, 66 * NG], f16, kind="ExternalInput")
    co_d = nc.dram_tensor("co", [I, D * O], codt, kind="ExternalInput")
    y_d = nc.dram_tensor("y_s", [BLOC, O], f32, kind="ExternalOutput")

    with tile.TileContext(nc) as tc:
        with (
            tc.tile_pool(name="const", bufs=1) as cpool,
            tc.tile_pool(name="work", bufs=2) as wpool,
            tc.tile_pool(name="qp", bufs=1, space="PSUM") as qpool,
            tc.tile_pool(name="sp", bufs=3, space="PSUM") as spool,
            tc.tile_pool(name="yp", bufs=1, space="PSUM") as ypool,
        ):
            # --- input DMAs.  sync (fast HWDGE): uqpq then co chunks 0,1.
            # gpsimd (SWDGE, +1.3us latency, but ws/xs get its first slots
            # so the co stream never delays their descriptor generation):
            # ws, xs, then co chunks 2,3. ---
            scr = cpool.tile([128, FD], f16)     # warm-up operand
            nc.gpsimd.memset(scr[:], 0.0)
            uqpq_t = cpool.tile([33, FD + 128], f16)
            nc.sync.dma_start(uqpq_t[:], uqpq_d.ap()[:])
            ws_t = cpool.tile([NPART, 66 * NG], f16)
            nc.gpsimd.dma_start(ws_t[:], ws_d.ap()[:])
            # trig tile: rows 0..112 sin/cos features, 112..120 xh, 120 ones
            trig = cpool.tile([NPART, FD], f16)
            nc.sync.dma_start(trig[NG * GROWS:NPART, :], xs_d.ap()[:])
            # co d-ranges alternate queues so each queue's first co transfer
            # carries early-d coefficients (per-queue DMA startup gaps would
            # otherwise starve the y consumption of mid-range d)
            co_t = cpool.tile([I, D * O], codt)
            dper = (D + CO_CHUNKS - 1) // CO_CHUNKS
            cch = [(c * dper, min(D, (c + 1) * dper)) for c in range(CO_CHUNKS)]
            for c in range(CO_CHUNKS):
                d0, d1 = cch[c]
                eng = nc.sync if c % 2 == 0 else nc.gpsimd
                eng.dma_start(co_t[:, O * d0:O * d1],
                              co_d.ap()[:, O * d0:O * d1])

            # --- PE warm-up: keep the tensor engine busy (at full array
            # power) through every wait so the p-state governor ramps to and
            # holds full clock.  All PE instructions are chained with
            # ordering-only deps so the scheduler cannot interleave them.
            # Warms dump into y_a, which the first real y matmul resets. ---
            from concourse.tile_rust import add_dep_helper
            y_a = ypool.tile([BLOC, O], f32, tag="ya")
            y_b = ypool.tile([BLOC, O], f32, tag="yb")
            pe_last = [None]

            def pe_order(instr):
                if pe_last[0] is not None:
                    add_dep_helper(instr.ins, pe_last[0].ins, sync=False,
                                   reason="PE issue order")
                pe_last[0] = instr
                return instr

            def warm(n):
                for _ in range(n):
                    pe_order(nc.tensor.matmul(y_a[:], scr[:, 0:BLOC],
                                              scr[:, 0:O],
                                              start=True, stop=True))

            warm(WARM[0])

            # --- phase matmul + range reduction + Sin, 2 pipelined chunks.
            # PSUM banks: q chunks and S tiles share one 3-deep ring of
            # 2-bank tiles (q banks recycle into the later S tiles); each
            # q chunk gets its own tile so no bank-level WAR stall. ---
            cw = FD // QCH
            dve_last = [None]

            def dve_order(instr):
                if dve_last[0] is not None:
                    add_dep_helper(instr.ins, dve_last[0].ins, sync=False,
                                   reason="DVE chunk order")
                dve_last[0] = instr
                return instr

            for h in range(QCH):
                cs = slice(h * cw, (h + 1) * cw)
                q = spool.tile([128, 776], f32, tag="S")
                pe_order(nc.tensor.matmul(q[0:NG * GROWS, 0:cw],
                                          uqpq_t[:, FD:FD + NG * GROWS],
                                          uqpq_t[:, cs],
                                          start=True, stop=True))
                f = wpool.tile([NG * GROWS, cw], f32, tag="f")
                qr = wpool.tile([NG * GROWS, cw], f32, tag="qr")
                dve_order(nc.vector.tensor_scalar(
                    qr[:], q[0:NG * GROWS, 0:cw], MAGIC, MAGIC,
                    op0=Alu.add, op1=Alu.subtract))
                dve_order(nc.vector.tensor_tensor(
                    f[:], q[0:NG * GROWS, 0:cw], qr[:],
                    op=Alu.subtract))
                nc.scalar.activation(trig[0:NG * GROWS, cs], f[:], Act.Sin,
                                     scale=TWO_PI)

            warm(WARM[1])

            # --- per 128-col slice j: series values + mask -> basis ---
            # S cols per group block of 66: [0..33) = z_d, [33..66) = S_d
            # bas col layout: m*D + d with m = 8j+g (host un-permutes rows)
            # matmul halves at cols 248/512 : bank-aligned dsts, adjacent
            # 528-col span so one Square/stt covers all 8 groups
            bas = cpool.tile([I, BLOC * D], f16)
            for j in range(4):
                if j == 2:
                    warm(WARM[2])
                S = spool.tile([128, 776], f32, tag="S")
                for h in range(2):
                    pe_order(nc.tensor.matmul(
                        S[:, 248 + 264 * h:248 + 264 * (h + 1)],
                        trig[:, 128 * j:128 * (j + 1)],
                        ws_t[:, 264 * h:264 * h + 264],
                        start=True, stop=True))
                Sv = S[:, 248:776].rearrange("p (g c) -> p g c", c=66)
                msq = wpool.tile([128, NG * D], f32, tag="msq")
                nc.scalar.activation(
                    msq[:].rearrange("p (g d) -> p g d", d=D),
                    Sv[:, :, 0:D], Act.Square)
                bj = bas[:, NG * D * j:NG * D * (j + 1)].rearrange(
                    "p (g d) -> p g d", d=D)
                mv = msq[:].rearrange("p (g d) -> p g d", d=D)
                # two stts per j (d < DSPLIT, d >= DSPLIT): the low-d half of
                # bas completes early so the y_a accumulation starts before
                # the high-d mask work is done
                for lo, hi in ((0, DSPLIT), (DSPLIT, D)):
                    nc.vector.scalar_tensor_tensor(
                        bj[:, :, lo:hi], in0=mv[:, :, lo:hi],
                        scalar=A_SUP * A_SUP, in1=Sv[:, :, D + lo:D + hi],
                        op0=Alu.is_le, op1=Alu.mult)

            warm(WARM[3])

            # --- final contraction over (i, d): two accumulators so the
            # low-d matmuls start as soon as the low-d bas halves land;
            # 1/CO_SCALE is folded into ws so the merge is a plain add ---
            basb = bas[:].rearrange("p (m c) -> p c m", c=D)
            for d in range(DSPLIT):
                pe_order(nc.tensor.matmul(y_a[:], basb[:, d, :],
                                          co_t[:, O * d:O * (d + 1)],
                                          start=(d == 0),
                                          stop=(d == DSPLIT - 1)))
            # y_a drains to SBUF while the y_b matmuls still run (only one
            # PSUM operand is allowed per DVE op anyway)
            y_sa = cpool.tile([BLOC, O], f32)
            nc.vector.tensor_scalar(y_sa[:], y_a[:], 1.0, None, op0=Alu.mult)
            for d in range(DSPLIT, D):
                pe_order(nc.tensor.matmul(y_b[:], basb[:, d, :],
                                          co_t[:, O * d:O * (d + 1)],
                                          start=(d == DSPLIT),
                                          stop=(d == D - 1)))
            y_s = cpool.tile([BLOC, O], f32)
            nc.vector.tensor_tensor(y_s[:], y_sa[:], y_b[:], op=Alu.add)
            nc.sync.dma_start(y_d.ap()[:], y_s[:])

    nc.compile()
    return nc


def _fup_coeffs():
    k = np.arange(1, NK + 1, dtype=np.float64)
    t = (np.pi / A_SUP) * k
    sinc = lambda z: np.sinc(z / np.pi)
    c = sinc(t / 2.0) ** N_ORDER
    for j in range(1, NPROD + 1):
        c = c * sinc(t / (2.0 ** j))
    return t, c


def _host_constants(compression, centers):
    comp = np.asarray(compression, np.float64)
    cent = np.asarray(centers, np.float64)
    assert comp.shape == (D,) and cent.shape == (D,)
    assert np.all(comp == CVAL), "kernel assumes compression == 16"
    assert np.allclose(cent, np.arange(D) / 16.0 - 1.0, atol=0, rtol=0), \
        "kernel assumes centers on the d/16-1 grid"

    t, c = _fup_coeffs()

    # per-partition phase constants theta/2pi split into fp16 hi+lo
    th = np.zeros(NG * GROWS, np.float64)
    off = np.zeros(NG * GROWS, np.float64)
    feat = np.concatenate([t * CVAL / (2 * np.pi)] * 2)   # cos rows, sin rows
    foff = np.concatenate([np.full(NK, 0.25), np.zeros(NK)])
    for g in range(NG):
        th[GROWS * g:GROWS * (g + 1)] = feat
        off[GROWS * g:GROWS * (g + 1)] = foff
    th_h = th.astype(F16).astype(np.float64)
    th_l = (th - th_h).astype(F16).astype(np.float64)
    pq = np.zeros((33, NG * GROWS), np.float64)
    for g in range(NG):
        ps = slice(GROWS * g, GROWS * (g + 1))
        pq[4 * g + 0, ps] = th_h[ps]
        pq[4 * g + 1, ps] = th_h[ps]
        pq[4 * g + 2, ps] = th_l[ps]
        pq[4 * g + 3, ps] = th_l[ps]
    pq[32, :] = off

    # feature -> (z_0..32, S_0..32) weights; phi_kd = t_k*((d mod 3)-16)
    ws = np.zeros((NPART, 66 * NG), np.float64)
    for g in range(NG):
        gc = 66 * g
        for dd in range(D):                        # z_d = 16*xh + (16 - d)
            ws[NG * GROWS + g, gc + dd] = CVAL
            ws[NG * GROWS + NG, gc + dd] = CVAL - dd
            phi = t * ((dd % 3) - CVAL)
            sc = A_SUP * CO_SCALE    # 1/CO_SCALE folded into the S columns
            ws[GROWS * g:GROWS * g + NK, gc + D + dd] = c * np.cos(phi) / sc
            ws[GROWS * g + NK:GROWS * (g + 1), gc + D + dd] = \
                c * np.sin(phi) / sc
            ws[NG * GROWS + NG, gc + D + dd] = 0.5 / sc
    return pq.astype(F16), ws.astype(F16)


# row m of device output corresponds to batch row b = 4*(m%8) + m//8
_PERM = np.array([8 * (b % 4) + b // 4 for b in range(BLOC)])


def _run(inputs, trace=False, **kw):
    global _PROG
    from concourse.bass_utils import run_bass_kernel_spmd

    if _PROG is None:
        _PROG = _build_program()
    nc = _PROG

    x = np.ascontiguousarray(np.asarray(inputs["x"], np.float32))
    coeffs = np.asarray(inputs["atomic_coeffs"], np.float32)
    pq, ws = _host_constants(inputs["compression"], inputs["centers"])
    cot = coeffs.transpose(0, 2, 1).astype(np.float64)
    if CO_FP8:
        import ml_dtypes
        co_scale = CO_QMAX / np.abs(cot).max()
        co = np.ascontiguousarray(
            (cot * co_scale).astype(ml_dtypes.float8_e3m4).reshape(I, D * O))
    else:
        co_scale = CO_SCALE
        co = np.ascontiguousarray(
            (cot * co_scale).astype(F16).reshape(I, D * O))
    y_rescale = np.float32(CO_SCALE / co_scale)

    in_maps = []
    for cid in range(NCORES):
        xflat = x[cid * BLOC:(cid + 1) * BLOC].reshape(ROWS)
        uh = xflat.astype(F16)
        ul = (xflat - uh.astype(np.float32)).astype(F16)
        uqpq = np.zeros((33, FD + 128), F16)
        for g in range(NG):
            es = slice(FD * g, FD * (g + 1))
            uqpq[4 * g + 0, 0:FD] = uh[es]
            uqpq[4 * g + 1, 0:FD] = ul[es]
            uqpq[4 * g + 2, 0:FD] = uh[es]
            uqpq[4 * g + 3, 0:FD] = ul[es]
        uqpq[32, 0:FD] = F16(1.0)
        uqpq[:, FD:FD + NG * GROWS] = pq
        xs = np.empty((NG + 1, FD), F16)
        for g in range(NG):
            xs[g] = uh[FD * g:FD * (g + 1)]
        xs[NG] = F16(1.0)
        in_maps.append({"uqpq": uqpq, "xs": xs, "ws": ws, "co": co})

    res = run_bass_kernel_spmd(nc, in_maps, core_ids=list(range(NCORES)),
                               trace=trace, **kw)
    y = np.concatenate([res.results[c]["y_s"][_PERM] for c in range(NCORES)],
                       axis=0)
    return (y * y_rescale).astype(np.float32, copy=False), res


def kernel(**inputs):
    y, _ = _run(inputs, trace=False)
    return y


# revision 49
# speedup vs baseline: 1.1805x; 1.1805x over previous
"""Trainium2 Bass kernel for nn_AtomicKANLayer (v8).

Math: y[b,o] = sum_{i,d} fupn((x[b,i]-centers[d])*compression[d]) * coeffs[i,o,d]
with fupn evaluated via its Fourier series.  Key structure exploited:

* The series coefficients decay superpolynomially: NK=7 terms leave 6.6e-4
  relative error (gate is 2e-2), so the trig features of EIGHT 512-element
  element-groups pack into 121 partitions (14 trig rows + 1 x row per group,
  one shared ones row) and every elementwise pass runs on a 512-wide free dim
  instead of 2048.
* With centers on the exact grid cent_d = d/16-1 and compression 16, the
  series argument z_d = u - d (u = 16x+16) is 3-PERIODIC in d: only three
  distinct series values S_c per element (c = d mod 3).  The per-(elem,d)
  basis is S_{d mod 3} * [z_d^2 <= a^2]; the d-replication of the three
  S_c values is folded into the ws matmul weights (group blocks of
  33 z cols + 33 S cols), keeping every elementwise AP <= 3D.

* The 2.1MB coefficient tensor is quantized to fp8e3 (e3m4, 4 mantissa
  bits) on the host, halving the HBM-bound coefficient stream that gates
  the output contraction.  Measured end-to-end relative error 1.24e-2
  (gate 2e-2); the host rescales y by CO_SCALE/co_scale afterwards.

The PE p-state governor only reaches full clock (~0.42ns/col) after several
microseconds of sustained full-array matmul power; K=1 matmuls do not count,
and the tile scheduler reorders same-engine work.  So every known PE wait
window is filled with full-K dummy matmuls into a scratch PSUM bank, with
all PE instructions chained by ordering-only deps, so the 33-matmul output
contraction runs at full clock almost immediately.  Warm counts are tuned
down from (10,9,1,5): the excess warms tripped the activity governor's
50%-duty cap (ham k=4/8 window) squarely across the y contraction.

Device pipeline per core (data-parallel over batch, 32 rows of B=256):
  DMA: sync queue: uq+pq first, then coeff chunks 0,1 and the y store;
  gpsimd queue: w_s, xs, coeff chunks 2,3 (small critical transfers are
  never queued behind the 2.2MB coeff stream).
  1. q[p,c] = theta_p/2pi * x + off_p  (exact split-fp16 matmul, K=33)  [PE]
  2. f = q - round(q) via fp32 magic-constant (ts add/sub, tt sub)     [DVE]
  3. trig[0:112] = Sin(2pi f) -> fp16, two 256-col chunks              [ACT]
  4. S_j = trig-colslice.T @ w_s  -> [128 i, 4 groups x (z_d, S_d)],
     two matmuls per j into bank-aligned halves of a 2-bank PSUM tile  [PE]
  5. msq = z^2                                                         [ACT]
     bas = (msq <= a^2) * S_d                                          [DVE]
  6. y = sum_d bas_d.T @ co_d (33 fp16 x fp8e3 matmuls, fp32 PSUM)      [PE]
  7. y_s = y_a + y_b -> DMA out (rows are (j,g)-permuted; host fixes)
"""
import sys

sys.path.insert(0, "/opt/trn_rl_repo")

import numpy as np

F16 = np.float16
B, I, O, D = 256, 128, 256, 33
NCORES = 8
BLOC = B // NCORES          # 32 batch rows per core
ROWS = BLOC * I             # 4096 flattened (b, i) elements per core
NG = 8                      # element groups packed along partitions
FD = ROWS // NG             # 512 free-dim columns per group
NK = 7                      # Fourier terms kept (of reference's 100)
GROWS = 2 * NK              # trig rows per group (cos+sin)
NPART = NG * GROWS + NG + 1  # 112 trig + 8 xh + 1 ones = 121
N_ORDER, NPROD = 1, 10
A_SUP = (N_ORDER + 2) / 2.0  # support half-width a = 1.5
CVAL = 16.0                  # compression (asserted on host)
MAGIC = float(np.float32(1.5 * 2**23))
TWO_PI = float(2 * np.pi)
QCH = 2                      # q -> sin chain chunks (of FD)
CO_CHUNKS = 4
CO_SCALE = 256.0             # bas = basis/CO_SCALE (keeps fp16 bas normal)
CO_FP8 = True                # co in fp8e3 (e3m4): halves the HBM-bound
                             # coeff stream; measured end-to-end 1.23e-2
CO_QMAX = 8.0                # quantize co with absmax -> 8.0
# PE warm-up dummies: (n_before_q, n_q_to_S, n_sin2_gap, n_S_to_y) 256-col
# full-K matmuls (the p-state governor ignores low-power work)
WARM = (8, 6, 1, 3)
DSPLIT = 16                  # y accumulator split point over d
USE_MOD = False

_PROG = None


def _build_program():
    import concourse.bacc as bacc
    import concourse.tile as tile
    from concourse import mybir

    f32 = mybir.dt.float32
    f16 = mybir.dt.float16
    codt = mybir.dt.float8e3 if CO_FP8 else f16
    Alu = mybir.AluOpType
    Act = mybir.ActivationFunctionType

    nc = bacc.Bacc("TRN2", target_bir_lowering=False, debug=False,
                   num_devices=NCORES)
    uqpq_d = nc.dram_tensor("uqpq", [33, FD + 128], f16, kind="ExternalInput")
    xs_d = nc.dram_tensor("xs", [NG + 1, FD], f16, kind="ExternalInput")
    ws_d = nc.dram_tensor("ws", [NPART, 66 * NG], f16, kind="ExternalInput")
    co_d = nc.dram_tensor("co", [I, D * O], codt, kind="ExternalInput")
    y_d = nc.dram_tensor("y_s", [BLOC, O], f32, kind="ExternalOutput")

    with tile.TileContext(nc) as tc:
        with (
            tc.tile_pool(name="const", bufs=1) as cpool,
            tc.tile_pool(name="work", bufs=2) as wpool,
            tc.tile_pool(name="qp", bufs=1, space="PSUM") as qpool,
            tc.tile_pool(name="sp", bufs=3, space="PSUM") as spool,
            tc.tile_pool(name="yp", bufs=1, space="PSUM") as ypool,
        ):
            # --- input DMAs.  sync (fast HWDGE): uqpq then co chunks 0,1.
            # gpsimd (SWDGE, +1.3us latency, but ws/xs get its first slots
            # so the co stream never delays their descriptor generation):
            # ws, xs, then co chunks 2,3. ---
            scr = cpool.tile([128, FD], f16)     # warm-up operand
            nc.gpsimd.memset(scr[:], 0.0)
            uqpq_t = cpool.tile([33, FD + 128], f16)
            nc.sync.dma_start(uqpq_t[:], uqpq_d.ap()[:])
            ws_t = cpool.tile([NPART, 66 * NG], f16)
            nc.gpsimd.dma_start(ws_t[:], ws_d.ap()[:])
            # trig tile: rows 0..112 sin/cos features, 112..120 xh, 120 ones
            trig = cpool.tile([NPART, FD], f16)
            nc.sync.dma_start(trig[NG * GROWS:NPART, :], xs_d.ap()[:])
            # co d-ranges alternate queues so each queue's first co transfer
            # carries early-d coefficients (per-queue DMA startup gaps would
            # otherwise starve the y consumption of mid-range d)
            co_t = cpool.tile([I, D * O], codt)
            dper = (D + CO_CHUNKS - 1) // CO_CHUNKS
            cch = [(c * dper, min(D, (c + 1) * dper)) for c in range(CO_CHUNKS)]
            for c in range(CO_CHUNKS):
                d0, d1 = cch[c]
                eng = nc.sync if c % 2 == 0 else nc.gpsimd
                eng.dma_start(co_t[:, O * d0:O * d1],
                              co_d.ap()[:, O * d0:O * d1])

            # --- PE warm-up: keep the tensor engine busy (at full array
            # power) through every wait so the p-state governor ramps to and
            # holds full clock.  All PE instructions are chained with
            # ordering-only deps so the scheduler cannot interleave them.
            # Warms dump into y_a, which the first real y matmul resets. ---
            from concourse.tile_rust import add_dep_helper
            y_a = ypool.tile([BLOC, O], f32, tag="ya")
            y_b = ypool.tile([BLOC, O], f32, tag="yb")
            pe_last = [None]

            def pe_order(instr):
                if pe_last[0] is not None:
                    add_dep_helper(instr.ins, pe_last[0].ins, sync=False,
                                   reason="PE issue order")
                pe_last[0] = instr
                return instr

            def warm(n):
                for _ in range(n):
                    pe_order(nc.tensor.matmul(y_a[:], scr[:, 0:BLOC],
                                              scr[:, 0:O],
                                              start=True, stop=True))

            warm(WARM[0])

            # --- phase matmul + range reduction + Sin, 2 pipelined chunks.
            # PSUM banks: q chunks and S tiles share one 3-deep ring of
            # 2-bank tiles (q banks recycle into the later S tiles); each
            # q chunk gets its own tile so no bank-level WAR stall. ---
            cw = FD // QCH
            dve_last = [None]

            def dve_order(instr):
                if dve_last[0] is not None:
                    add_dep_helper(instr.ins, dve_last[0].ins, sync=False,
                                   reason="DVE chunk order")
                dve_last[0] = instr
                return instr

            for h in range(QCH):
                cs = slice(h * cw, (h + 1) * cw)
                q = spool.tile([128, 776], f32, tag="S")
                pe_order(nc.tensor.matmul(q[0:NG * GROWS, 0:cw],
                                          uqpq_t[:, FD:FD + NG * GROWS],
                                          uqpq_t[:, cs],
                                          start=True, stop=True))
                f = wpool.tile([NG * GROWS, cw], f32, tag="f")
                qr = wpool.tile([NG * GROWS, cw], f32, tag="qr")
                dve_order(nc.vector.tensor_scalar(
                    qr[:], q[0:NG * GROWS, 0:cw], MAGIC, MAGIC,
                    op0=Alu.add, op1=Alu.subtract))
                dve_order(nc.vector.tensor_tensor(
                    f[:], q[0:NG * GROWS, 0:cw], qr[:],
                    op=Alu.subtract))
                nc.scalar.activation(trig[0:NG * GROWS, cs], f[:], Act.Sin,
                                     scale=TWO_PI)

            warm(WARM[1])

            # --- per 128-col slice j: series values + mask -> basis ---
            # S cols per group block of 66: [0..33) = z_d, [33..66) = S_d
            # bas col layout: m*D + d with m = 8j+g (host un-permutes rows)
            # matmul halves at cols 248/512 : bank-aligned dsts, adjacent
            # 528-col span so one Square/stt covers all 8 groups
            bas = cpool.tile([I, BLOC * D], f16)
            for j in range(4):
                if j == 2:
                    warm(WARM[2])
                S = spool.tile([128, 776], f32, tag="S")
                for h in range(2):
                    pe_order(nc.tensor.matmul(
                        S[:, 248 + 264 * h:248 + 264 * (h + 1)],
                        trig[:, 128 * j:128 * (j + 1)],
                        ws_t[:, 264 * h:264 * h + 264],
                        start=True, stop=True))
                Sv = S[:, 248:776].rearrange("p (g c) -> p g c", c=66)
                msq = wpool.tile([128, NG * D], f32, tag="msq")
                nc.scalar.activation(
                    msq[:].rearrange("p (g d) -> p g d", d=D),
                    Sv[:, :, 0:D], Act.Square)
                bj = bas[:, NG * D * j:NG * D * (j + 1)].rearrange(
                    "p (g d) -> p g d", d=D)
                mv = msq[:].rearrange("p (g d) -> p g d", d=D)
                # two stts per j (d < DSPLIT, d >= DSPLIT): the low-d half of
                # bas completes early so the y_a accumulation starts before
                # the high-d mask work is done
                for lo, hi in ((0, DSPLIT), (DSPLIT, D)):
                    nc.vector.scalar_tensor_tensor(
                        bj[:, :, lo:hi], in0=mv[:, :, lo:hi],
                        scalar=A_SUP * A_SUP, in1=Sv[:, :, D + lo:D + hi],
                        op0=Alu.is_le, op1=Alu.mult)

            warm(WARM[3])

            # --- final contraction over (i, d): two accumulators so the
            # low-d matmuls start as soon as the low-d bas halves land;
            # 1/CO_SCALE is folded into ws so the merge is a plain add ---
            basb = bas[:].rearrange("p (m c) -> p c m", c=D)
            for d in range(DSPLIT):
                pe_order(nc.tensor.matmul(y_a[:], basb[:, d, :],
                                          co_t[:, O * d:O * (d + 1)],
                                          start=(d == 0),
                                          stop=(d == DSPLIT - 1)))
            # y_a drains to SBUF while the y_b matmuls still run (only one
            # PSUM operand is allowed per DVE op anyway)
            y_sa = cpool.tile([BLOC, O], f32)
            nc.vector.tensor_scalar(y_sa[:], y_a[:], 1.0, None, op0=Alu.mult)
            for d in range(DSPLIT, D):
                pe_order(nc.tensor.matmul(y_b[:], basb[:, d, :],
                                          co_t[:, O * d:O * (d + 1)],
                                          start=(d == DSPLIT),
                                          stop=(d == D - 1)))
            y_s = cpool.tile([BLOC, O], f32)
            nc.vector.tensor_tensor(y_s[:], y_sa[:], y_b[:], op=Alu.add)
            nc.sync.dma_start(y_d.ap()[:], y_s[:])

    nc.compile()
    return nc


def _fup_coeffs():
    k = np.arange(1, NK + 1, dtype=np.float64)
    t = (np.pi / A_SUP) * k
    sinc = lambda z: np.sinc(z / np.pi)
    c = sinc(t / 2.0) ** N_ORDER
    for j in range(1, NPROD + 1):
        c = c * sinc(t / (2.0 ** j))
    return t, c


def _host_constants(compression, centers):
    comp = np.asarray(compression, np.float64)
    cent = np.asarray(centers, np.float64)
    assert comp.shape == (D,) and cent.shape == (D,)
    assert np.all(comp == CVAL), "kernel assumes compression == 16"
    assert np.allclose(cent, np.arange(D) / 16.0 - 1.0, atol=0, rtol=0), \
        "kernel assumes centers on the d/16-1 grid"

    t, c = _fup_coeffs()

    # per-partition phase constants theta/2pi split into fp16 hi+lo
    th = np.zeros(NG * GROWS, np.float64)
    off = np.zeros(NG * GROWS, np.float64)
    feat = np.concatenate([t * CVAL / (2 * np.pi)] * 2)   # cos rows, sin rows
    foff = np.concatenate([np.full(NK, 0.25), np.zeros(NK)])
    for g in range(NG):
        th[GROWS * g:GROWS * (g + 1)] = feat
        off[GROWS * g:GROWS * (g + 1)] = foff
    th_h = th.astype(F16).astype(np.float64)
    th_l = (th - th_h).astype(F16).astype(np.float64)
    pq = np.zeros((33, NG * GROWS), np.float64)
    for g in range(NG):
        ps = slice(GROWS * g, GROWS * (g + 1))
        pq[4 * g + 0, ps] = th_h[ps]
        pq[4 * g + 1, ps] = th_h[ps]
        pq[4 * g + 2, ps] = th_l[ps]
        pq[4 * g + 3, ps] = th_l[ps]
    pq[32, :] = off

    # feature -> (z_0..32, S_0..32) weights; phi_kd = t_k*((d mod 3)-16)
    ws = np.zeros((NPART, 66 * NG), np.float64)
    for g in range(NG):
        gc = 66 * g
        for dd in range(D):                        # z_d = 16*xh + (16 - d)
            ws[NG * GROWS + g, gc + dd] = CVAL
            ws[NG * GROWS + NG, gc + dd] = CVAL - dd
            phi = t * ((dd % 3) - CVAL)
            sc = A_SUP * CO_SCALE    # 1/CO_SCALE folded into the S columns
            ws[GROWS * g:GROWS * g + NK, gc + D + dd] = c * np.cos(phi) / sc
            ws[GROWS * g + NK:GROWS * (g + 1), gc + D + dd] = \
                c * np.sin(phi) / sc
            ws[NG * GROWS + NG, gc + D + dd] = 0.5 / sc
    return pq.astype(F16), ws.astype(F16)


# row m of device output corresponds to batch row b = 4*(m%8) + m//8
_PERM = np.array([8 * (b % 4) + b // 4 for b in range(BLOC)])


def _run(inputs, trace=False, **kw):
    global _PROG
    from concourse.bass_utils import run_bass_kernel_spmd

    if _PROG is None:
        _PROG = _build_program()
    nc = _PROG

    x = np.ascontiguousarray(np.asarray(inputs["x"], np.float32))
    coeffs = np.asarray(inputs["atomic_coeffs"], np.float32)
    pq, ws = _host_constants(inputs["compression"], inputs["centers"])
    cot = coeffs.transpose(0, 2, 1).astype(np.float64)
    if CO_FP8:
        import ml_dtypes
        co_scale = CO_QMAX / np.abs(cot).max()
        co = np.ascontiguousarray(
            (cot * co_scale).astype(ml_dtypes.float8_e3m4).reshape(I, D * O))
    else:
        co_scale = CO_SCALE
        co = np.ascontiguousarray(
            (cot * co_scale).astype(F16).reshape(I, D * O))
    y_rescale = np.float32(CO_SCALE / co_scale)

    in_maps = []
    for cid in range(NCORES):
        xflat = x[cid * BLOC:(cid + 1) * BLOC].reshape(ROWS)
        uh = xflat.astype(F16)
        ul = (xflat - uh.astype(np.float32)).astype(F16)
        uqpq = np.zeros((33, FD + 128), F16)
        for g in range(NG):
            es = slice(FD * g, FD * (g + 1))
            uqpq[4 * g + 0, 0:FD] = uh[es]
            uqpq[4 * g + 1, 0:FD] = ul[es]
            uqpq[4 * g + 2, 0:FD] = uh[es]
            uqpq[4 * g + 3, 0:FD] = ul[es]
        uqpq[32, 0:FD] = F16(1.0)
        uqpq[:, FD:FD + NG * GROWS] = pq
        xs = np.empty((NG + 1, FD), F16)
        for g in range(NG):
            xs[g] = uh[FD * g:FD * (g + 1)]
        xs[NG] = F16(1.0)
        in_maps.append({"uqpq": uqpq, "xs": xs, "ws": ws, "co": co})

    res = run_bass_kernel_spmd(nc, in_maps, core_ids=list(range(NCORES)),
                               trace=trace, **kw)
    y = np.concatenate([res.results[c]["y_s"][_PERM] for c in range(NCORES)],
                       axis=0)
    return (y * y_rescale).astype(np.float32, copy=False), res


def kernel(**inputs):
    y, _ = _run(inputs, trace=False)
    return y
